# revision 33
# baseline (speedup 1.0000x reference)
"""CrossLayerRouter (noisy top-k MoE routing) Trainium2 kernel.

Strategy (data-parallel over 8 NeuronCores, 4096 tokens each):
  - Host splits x into bf16 hi/lo halves and pre-transposes per-core shards
    to [2048, 4096] so the contraction dim lands on SBUF partitions.
  - Device: out[e, t] = W.T @ xT via 3 bf16 matmul terms
    (Wh.x_h + Wl.x_h + Wh.x_l) accumulated in fp32 PSUM — ~1e-5 abs match
    to the fp32 reference. Skip logit = 1-term bf16 matmul (sigmoid is
    error-tolerant).
  - PE transposes the [128e, 512t] result back to token-major blocks,
    DVE adds biases, ACT computes softplus as Ln(exp(z)+1) (Exp+Ln live in
    the same activation table), DVE Max8/MaxIndex produce the exact
    stable top-8 (descending, ties -> lowest index, matching
    jax.lax.top_k), and gates come from exp(v - m0) masked by v >= t8 and
    normalized with a fused scalar_tensor_tensor + accum reduction.
  - Outputs are stored partition-major [p, J, e] so every DMA runs with
    1KB+ contiguous bursts; the host untransposes when reassembling.
"""
import sys

if "/opt/trn_rl_repo" not in sys.path:
    sys.path.insert(0, "/opt/trn_rl_repo")

import numpy as np
import ml_dtypes

import concourse.bacc as bacc
import concourse.tile as tile
import concourse.mybir as mybir
import concourse.bass_utils as bass_utils

F32 = mybir.dt.float32
BF16 = mybir.dt.bfloat16
I32 = mybir.dt.int32
U32 = mybir.dt.uint32
AF = mybir.ActivationFunctionType
OP = mybir.AluOpType

N_CORES = 8
D = 2048            # embed dim
E = 128             # router(64) + noise(64) experts, concatenated
TOPK = 8
T_CORE = 4096       # tokens per core
G_TOK = 512         # tokens per group (one PSUM bank)
N_GROUPS = T_CORE // G_TOK          # 8
N_BLK = G_TOK // 128                # 4 transpose blocks per group
N_J = T_CORE // 128                 # 32 token blocks per core
KC = D // 128                       # 16 contraction chunks

BF = ml_dtypes.bfloat16

_PROGRAM = None   # (nc,) cached compiled program
LAST_RESULTS = None


def _dedup_ldweights(nc):
    """Walrus runs with --enable-ldw-opt=false, so every LDWEIGHTS
    serializes with its matmul (~+120ns each here). Tile legalization
    emits one LDWEIGHTS per matmul even when consecutive matmuls use the
    identical stationary operand; the PE array still holds those weights,
    so the duplicate loads can be dropped (their waits move onto the
    matmul that followed)."""
    PE = mybir.EngineType.PE
    removed = 0
    for fn in nc.m.functions:
        for blk in fn.blocks:
            insts = list(blk.instructions)
            keep = []
            last_sig = None
            pending = None
            for ins in insts:
                if getattr(ins, "engine", None) != PE:
                    keep.append(ins)
                    continue
                tn = type(ins).__name__
                if tn == "InstLdweights":
                    ap = ins.ins[0]
                    sig = (getattr(ap, "memref", None),
                           getattr(ap, "offset", None),
                           str(getattr(ap, "ap", None)),
                           str(getattr(ap, "dtype", None)),
                           ins.is_transpose, ins.perf_mode,
                           str(getattr(ins, "tile_position", None)),
                           str(getattr(ins, "tile_size", None)))
                    if sig == last_sig and sig[0] is not None:
                        si = ins.sync_info
                        if si is not None and (list(si.on_wait) or
                                               list(si.on_update)):
                            pending = si
                        removed += 1
                        continue
                    last_sig = sig
                    keep.append(ins)
                elif tn == "InstMatmult":
                    if pending is not None:
                        cur = ins.sync_info
                        cw = list(cur.on_wait) if cur is not None else []
                        cu = list(cur.on_update) if cur is not None else []
                        ins.sync_info = mybir.SyncInfo(
                            on_wait=cw + list(pending.on_wait),
                            on_update=cu + list(pending.on_update))
                        pending = None
                    keep.append(ins)
                else:
                    last_sig = None
                    keep.append(ins)
            assert pending is None
            if len(keep) != len(insts):
                blk.instructions = keep
    return removed


def _build_program():
    nc = bacc.Bacc("TRN2", target_bir_lowering=False, debug=False,
                   num_devices=N_CORES)

    def din(name, shape, dt):
        return nc.dram_tensor(name, list(shape), dt, kind="ExternalInput").ap()

    def dout(name, shape, dt):
        return nc.dram_tensor(name, list(shape), dt, kind="ExternalOutput").ap()

    i_xh = din("xh", [D, T_CORE], BF16)       # bf16 hi of x, transposed
    i_xl = din("xl", [D, T_CORE], BF16)       # bf16 lo of x, transposed
    i_eps = din("eps", [128, N_J, 64], F32)   # eps, partition-major
    i_wh = din("wh", [D, E], BF16)
    i_wl = din("wl", [D, E], BF16)
    i_ws = din("ws", [D, 1], BF16)            # skip weight (bf16 hi)
    i_bias = din("bias", [128, N_BLK, E], F32)  # router|noise bias, replicated
    i_ident = din("ident", [128, 128], F32)
    i_nbsk = din("nbsk", [128, 1], F32)       # -b_skip, replicated

    o_rout = dout("o_rout", [128, N_J, 64], F32)          # [p, J, e]
    o_idx = dout("o_idx", [128, N_J, TOPK], I32)          # [p, J, k]
    o_skip = dout("o_skip", [128, N_GROUPS, N_BLK], F32)  # [p, g, f]

    NSUB = 4
    KSUB = KC // NSUB

    with tile.TileContext(nc) as tc:
        with tc.tile_pool(name="cst", bufs=1) as cst, \
             tc.tile_pool(name="xin", bufs=3) as xin, \
             tc.tile_pool(name="wrk", bufs=2) as wrk, \
             tc.tile_pool(name="drm", bufs=1, space="DRAM") as drm, \
             tc.tile_pool(name="mps", bufs=3, space="PSUM") as mps, \
             tc.tile_pool(name="tps", bufs=2, space="PSUM") as tps, \
             tc.tile_pool(name="sps", bufs=2, space="PSUM") as sps:

            def load_group(g, interleave=None):
                """Queue this group's x hi/lo sub-slabs + eps on the SP
                HWDGE queue (inputs only — outputs go via the ACT queue so
                they never block the x stream)."""
                t0 = g * G_TOK
                xh_t, xl_t = [], []
                for s_ in range(NSUB):
                    k0 = s_ * KSUB
                    xh_s = xin.tile([128, KSUB, G_TOK], BF16,
                                    name=f"xh_s{s_}", tag=f"xh_s{s_}")
                    nc.sync.dma_start(
                        xh_s[:], i_xh[k0 * 128:(k0 + KSUB) * 128,
                                      t0:t0 + G_TOK].rearrange(
                            "(kc p) t -> p kc t", p=128))
                    xl_s = xin.tile([128, KSUB, G_TOK], BF16,
                                    name=f"xl_s{s_}", tag=f"xl_s{s_}")
                    nc.sync.dma_start(
                        xl_s[:], i_xl[k0 * 128:(k0 + KSUB) * 128,
                                      t0:t0 + G_TOK].rearrange(
                            "(kc p) t -> p kc t", p=128))
                    xh_t.append(xh_s)
                    xl_t.append(xl_s)
                    if s_ == 0 and interleave is not None:
                        interleave()
                eps_g = xin.tile([128, N_BLK, 64], F32, name="eps_g",
                                 tag="eps_g")
                nc.gpsimd.dma_start(eps_g[:], i_eps[:, 4 * g:4 * g + N_BLK, :])
                return xh_t, xl_t, eps_g

            # ---- constants; only wh is needed before the first matmul, so
            # group 0's x DMAs are queued right after it ----
            wh_sb = cst.tile([128, KC, E], BF16, name="wh_sb")

            def _load_wh_rest():
                for s_ in range(1, NSUB):
                    k0 = s_ * KSUB
                    nc.sync.dma_start(
                        wh_sb[:, k0:k0 + KSUB, :],
                        i_wh[k0 * 128:(k0 + KSUB) * 128, :].rearrange(
                            "(kc p) e -> p kc e", p=128))

            nc.sync.dma_start(
                wh_sb[:, 0:KSUB, :],
                i_wh[0:KSUB * 128, :].rearrange("(kc p) e -> p kc e", p=128))
            g0_tiles = load_group(0, interleave=_load_wh_rest)
            wl_sb = cst.tile([128, KC, E], BF16, name="wl_sb")
            nc.gpsimd.dma_start(wl_sb[:],
                                i_wl.rearrange("(kc p) e -> p kc e", p=128))
            ws_sb = cst.tile([128, KC, 1], BF16, name="ws_sb")
            nc.gpsimd.dma_start(ws_sb[:],
                                i_ws.rearrange("(kc p) e -> p kc e", p=128))
            bias_sb = cst.tile([128, N_BLK, E], F32, name="bias_sb")
            nc.gpsimd.dma_start(bias_sb[:], i_bias)
            id_sb = cst.tile([128, 128], F32, name="id_sb")
            nc.gpsimd.dma_start(id_sb[:], i_ident)
            nbsk_sb = cst.tile([128, 1], F32, name="nbsk_sb")
            nc.gpsimd.dma_start(nbsk_sb[:], i_nbsk)

            skz_dram = drm.tile([N_GROUPS, G_TOK], F32, name="skz_dram")

            for g in range(N_GROUPS):
                xh_t, xl_t, eps_g = g0_tiles if g == 0 else load_group(g)

                # ---- matmuls: out[e, t] over 3 bf16 terms, kc-major with
                # (wh:xh, wh:xl) adjacent so the duplicate LDWEIGHTS of the
                # second matmul is dropped by _dedup_ldweights ----
                mm_ps = mps.tile([128, G_TOK], F32, name="mm_ps", tag="mm_ps")
                sk_ps = sps.tile([128, G_TOK], F32, name="sk_ps", tag="sk_ps")
                n_mm = 3 * KC
                i_mm = 0
                for kc in range(KC):
                    xh_kc = xh_t[kc // KSUB][:, kc % KSUB, :]
                    xl_kc = xl_t[kc // KSUB][:, kc % KSUB, :]
                    for w_t, x_t in ((wh_sb, xh_kc), (wh_sb, xl_kc)):
                        nc.tensor.matmul(mm_ps[:], lhsT=w_t[:, kc, :], rhs=x_t,
                                         start=(i_mm == 0),
                                         stop=(i_mm == n_mm - 1))
                        i_mm += 1
                # wl pass after all wh pairs: wl_sb is queued behind group
                # 0's x slabs, so the first ~32 matmuls must not need it
                for kc in range(KC):
                    xh_kc = xh_t[kc // KSUB][:, kc % KSUB, :]
                    nc.tensor.matmul(mm_ps[:], lhsT=wl_sb[:, kc, :], rhs=xh_kc,
                                     start=(i_mm == 0),
                                     stop=(i_mm == n_mm - 1))
                    i_mm += 1
                # skip logit (1-term bf16), column-tiled: four concurrent
                # M=1 matmuls in distinct 32-col PE strips, one per
                # 128-token block, so the N=512 stream takes ~N/4 cycles
                for kc in range(KC):
                    xh_kc = xh_t[kc // KSUB][:, kc % KSUB, :]
                    for j in range(N_BLK):
                        nc.tensor.matmul(
                            sk_ps[32 * j:32 * j + 1, j * 128:(j + 1) * 128],
                            lhsT=ws_sb[:, kc, :],
                            rhs=xh_kc[:, j * 128:(j + 1) * 128],
                            start=(kc == 0), stop=(kc == KC - 1),
                            tile_position=(0, 32 * j))

                # ---- PSUM -> SBUF, then PE-transpose back to [t, e] ----
                res_sb = wrk.tile([128, G_TOK], F32, name="res_sb",
                                  tag="res_sb")
                nc.scalar.activation(res_sb[:], mm_ps[:], AF.Copy)
                skg_sb = wrk.tile([1, G_TOK], F32, name="skg_sb", tag="skg_sb")
                for j in range(N_BLK):
                    nc.scalar.activation(
                        skg_sb[:, j * 128:(j + 1) * 128],
                        sk_ps[32 * j:32 * j + 1, j * 128:(j + 1) * 128],
                        AF.Copy)

                # ---- skip prob for this group: bounce [1, 512] through
                # DRAM to [128, 4] lanes, then sigmoid = 1/(1+exp(-z-b)) ----
                nc.gpsimd.dma_start(skz_dram[g:g + 1, :], skg_sb[:])
                sk4_sb = wrk.tile([128, N_BLK], F32, name="sk4_sb",
                                  tag="sk4_sb")
                nc.gpsimd.dma_start(
                    sk4_sb[:],
                    skz_dram[g:g + 1, :].rearrange("o (p f) -> (o p) f",
                                                   p=128))
                ske_sb = wrk.tile([128, N_BLK], F32, name="ske_sb",
                                  tag="ske_sb")
                nc.scalar.activation(ske_sb[:], sk4_sb[:], AF.Exp, scale=-1.0,
                                     bias=nbsk_sb[:])
                nc.vector.tensor_scalar_add(ske_sb[:], ske_sb[:], 1.0)
                skp_sb = wrk.tile([128, N_BLK], F32, name="skp_sb",
                                  tag="skp_sb")
                nc.vector.reciprocal(skp_sb[:], ske_sb[:])
                nc.gpsimd.dma_start(o_skip[:, g, :], skp_sb[:])


                tr_ps = tps.tile([128, N_BLK, 128], F32, name="tr_ps",
                                 tag="tr_ps")
                for j in range(N_BLK):
                    nc.tensor.transpose(tr_ps[:, j, :],
                                        res_sb[:, j * 128:(j + 1) * 128],
                                        id_sb[:])

                # ---- bias add (also moves PSUM -> SBUF) ----
                z_sb = wrk.tile([128, N_BLK, E], F32, name="z_sb", tag="z_sb")
                nc.vector.tensor_add(z_sb[:], tr_ps[:], bias_sb[:])

                # ---- softplus(noise) = Ln(exp(z) + 1) ----
                et_sb = wrk.tile([128, N_BLK, 64], F32, name="et_sb",
                                 tag="et_sb")
                nc.scalar.activation(et_sb[:], z_sb[:, :, 64:128], AF.Exp)
                sp_sb = wrk.tile([128, N_BLK, 64], F32, name="sp_sb",
                                 tag="sp_sb")
                nc.scalar.activation(sp_sb[:], et_sb[:], AF.Ln, bias=1.0)

                # ---- noisy = logits + eps * softplus ----
                ns_sb = wrk.tile([128, N_BLK, 64], F32, name="ns_sb",
                                 tag="ns_sb")
                nc.vector.tensor_mul(ns_sb[:], eps_g[:], sp_sb[:])
                nc.vector.tensor_add(ns_sb[:], ns_sb[:], z_sb[:, :, 0:64])

                # ---- stable top-8 (descending values + indices) ----
                mx_sb = wrk.tile([128, N_BLK, TOPK], F32, name="mx_sb",
                                 tag="mx_sb")
                ix_sb = wrk.tile([128, N_BLK, TOPK], I32, name="ix_sb",
                                 tag="ix_sb")
                for j in range(N_BLK):
                    nc.vector.max(mx_sb[:, j, :], ns_sb[:, j, :])
                    nc.vector.max_index(ix_sb[:, j, :].bitcast(U32),
                                        mx_sb[:, j, :], ns_sb[:, j, :])
                nc.gpsimd.dma_start(o_idx[:, 4 * g:4 * g + N_BLK, :], ix_sb[:])

                # ---- gates: exp(v - m0) masked by v >= t8, normalized ----
                nm0_sb = wrk.tile([128, N_BLK], F32, name="nm0_sb",
                                  tag="nm0_sb")
                nc.vector.tensor_scalar_mul(nm0_sb[:], mx_sb[:, :, 0], -1.0)
                ex_sb = wrk.tile([128, N_BLK, 64], F32, name="ex_sb",
                                 tag="ex_sb")
                for j in range(N_BLK):
                    nc.scalar.activation(ex_sb[:, j, :], ns_sb[:, j, :],
                                         AF.Exp, bias=nm0_sb[:, j:j + 1])
                rs_sb = wrk.tile([128, N_BLK], F32, name="rs_sb", tag="rs_sb")
                r_sb = wrk.tile([128, N_BLK, 64], F32, name="r_sb", tag="r_sb")
                for j in range(N_BLK):
                    nc.vector.scalar_tensor_tensor(
                        r_sb[:, j, :], ns_sb[:, j, :], mx_sb[:, j, 7:8],
                        ex_sb[:, j, :], op0=OP.is_ge, op1=OP.mult,
                        accum_out=rs_sb[:, j:j + 1])
                ri_sb = wrk.tile([128, N_BLK], F32, name="ri_sb", tag="ri_sb")
                nc.vector.reciprocal(ri_sb[:], rs_sb[:])
                rg_sb = wrk.tile([128, N_BLK, 64], F32, name="rg_sb",
                                 tag="rg_sb")
                for j in range(N_BLK):
                    nc.vector.tensor_scalar_mul(
                        rg_sb[:, j, :], r_sb[:, j, :], ri_sb[:, j:j + 1])
                nc.gpsimd.dma_start(o_rout[:, 4 * g:4 * g + N_BLK, :],
                                    rg_sb[:])
    # Pin every activation to act table 6 (natural_log_exp_and_others: has
    # Copy+Exp+Ln) so a single ACT_TABLE_LOAD serves the whole kernel. The
    # default chooser pairs Exp with table 0 and Ln with table 6 and
    # reloads ~1.3us on every switch. Indices must stay aligned with
    # act_info.json, so earlier entries are emptied rather than removed.
    import concourse.bacc as _bacc_mod
    from concourse.hw_specs import get_activation_tables as _gat
    real = list(_gat("gen3").items())
    pinned = {}
    for i, (name, funcs) in enumerate(real):
        pinned[name] = funcs if name == "natural_log_exp_and_others" else set()
    orig_gat = _bacc_mod.get_activation_tables
    _bacc_mod.get_activation_tables = lambda arch: pinned
    try:
        nc.compile()
    finally:
        _bacc_mod.get_activation_tables = orig_gat
    n_removed = _dedup_ldweights(nc)
    assert n_removed >= N_GROUPS * KC * 3 // 4, n_removed
    return nc


def _get_program():
    global _PROGRAM
    if _PROGRAM is None:
        _PROGRAM = _build_program()
    return _PROGRAM


def kernel(x, eps, w_router, b_router, w_noise, b_noise, w_skip, b_skip):
    nc = _get_program()

    x = np.ascontiguousarray(np.asarray(x, dtype=np.float32)).reshape(-1, D)
    eps = np.ascontiguousarray(np.asarray(eps, dtype=np.float32)).reshape(-1, 64)
    n_tok = x.shape[0]
    assert n_tok == N_CORES * T_CORE

    W = np.concatenate([np.asarray(w_router, np.float32),
                        np.asarray(w_noise, np.float32)], axis=1)
    wh = W.astype(BF)
    wl = (W - wh.astype(np.float32)).astype(BF)
    ws = np.asarray(w_skip, np.float32).astype(BF)
    bias_cat = np.concatenate([np.asarray(b_router, np.float32),
                               np.asarray(b_noise, np.float32)])
    bias_rep = np.broadcast_to(bias_cat, (128, N_BLK, E)).copy()
    ident = np.eye(128, dtype=np.float32)
    nbsk = np.full((128, 1), -float(np.asarray(b_skip).reshape(())), np.float32)

    xh_full = x.astype(BF)
    xl_full = (x - xh_full.astype(np.float32)).astype(BF)

    in_maps = []
    for c in range(N_CORES):
        s = slice(c * T_CORE, (c + 1) * T_CORE)
        in_maps.append({
            "xh": np.ascontiguousarray(xh_full[s].T),
            "xl": np.ascontiguousarray(xl_full[s].T),
            # eps token t = J*128 + p  ->  [p, J, e]
            "eps": np.ascontiguousarray(
                eps[s].reshape(N_J, 128, 64).transpose(1, 0, 2)),
            "wh": wh, "wl": wl, "ws": ws,
            "bias": bias_rep, "ident": ident, "nbsk": nbsk,
        })

    res = bass_utils.run_bass_kernel_spmd(nc, in_maps,
                                          core_ids=list(range(N_CORES)))
    global LAST_RESULTS
    LAST_RESULTS = res

    router = np.empty((n_tok, 64), np.float32)
    indices = np.empty((n_tok, TOPK), np.int32)
    skip = np.empty((n_tok,), np.float32)
    for c in range(N_CORES):
        out = res.results[c]
        s = slice(c * T_CORE, (c + 1) * T_CORE)
        # [p, J, ...] -> token t = J*128 + p
        router[s] = out["o_rout"].transpose(1, 0, 2).reshape(T_CORE, 64)
        indices[s] = out["o_idx"].transpose(1, 0, 2).reshape(T_CORE, TOPK)
        # skip output is [p, g, f] with token = g*512 + p*4 + f
        skip[s] = out["o_skip"].transpose(1, 0, 2).reshape(T_CORE)

    B, S = 4, 8192
    return (router.reshape(B, S, 64), indices.reshape(B, S, TOPK),
            skip.reshape(B, S, 1).astype(np.float32))


# revision 34
# speedup vs baseline: 1.0423x; 1.0423x over previous
"""CrossLayerRouter (noisy top-k MoE routing) Trainium2 kernel.

Strategy (data-parallel over 8 NeuronCores, 4096 tokens each):
  - Host splits x into bf16 hi/lo halves and pre-transposes per-core shards
    to [2048, 4096] so the contraction dim lands on SBUF partitions.
  - Device: out[e, t] = W.T @ xT via 3 bf16 matmul terms
    (Wh.x_h + Wl.x_h + Wh.x_l) accumulated in fp32 PSUM — ~1e-5 abs match
    to the fp32 reference. Skip logit = 1-term bf16 matmul (sigmoid is
    error-tolerant).
  - PE transposes the [128e, 512t] result back to token-major blocks,
    DVE adds biases, ACT computes softplus as Ln(exp(z)+1) (Exp+Ln live in
    the same activation table), DVE Max8/MaxIndex produce the exact
    stable top-8 (descending, ties -> lowest index, matching
    jax.lax.top_k), and gates come from exp(v - m0) masked by v >= t8 and
    normalized with a fused scalar_tensor_tensor + accum reduction.
  - Outputs are stored partition-major [p, J, e] so every DMA runs with
    1KB+ contiguous bursts; the host untransposes when reassembling.
"""
import sys

if "/opt/trn_rl_repo" not in sys.path:
    sys.path.insert(0, "/opt/trn_rl_repo")

import numpy as np
import ml_dtypes

import concourse.bacc as bacc
import concourse.tile as tile
import concourse.mybir as mybir
import concourse.bass_utils as bass_utils

F32 = mybir.dt.float32
BF16 = mybir.dt.bfloat16
I32 = mybir.dt.int32
U32 = mybir.dt.uint32
AF = mybir.ActivationFunctionType
OP = mybir.AluOpType

N_CORES = 8
D = 2048            # embed dim
E = 128             # router(64) + noise(64) experts, concatenated
TOPK = 8
T_CORE = 4096       # tokens per core
G_TOK = 512         # tokens per group (one PSUM bank)
N_GROUPS = T_CORE // G_TOK          # 8
N_BLK = G_TOK // 128                # 4 transpose blocks per group
N_J = T_CORE // 128                 # 32 token blocks per core
KC = D // 128                       # 16 contraction chunks

BF = ml_dtypes.bfloat16

_PROGRAM = None   # (nc,) cached compiled program
LAST_RESULTS = None


def _dedup_ldweights(nc):
    """Walrus runs with --enable-ldw-opt=false, so every LDWEIGHTS
    serializes with its matmul (~+120ns each here). Tile legalization
    emits one LDWEIGHTS per matmul even when consecutive matmuls use the
    identical stationary operand; the PE array still holds those weights,
    so the duplicate loads can be dropped (their waits move onto the
    matmul that followed)."""
    PE = mybir.EngineType.PE
    removed = 0
    for fn in nc.m.functions:
        for blk in fn.blocks:
            insts = list(blk.instructions)
            keep = []
            last_sig = None
            pending = None
            for ins in insts:
                if getattr(ins, "engine", None) != PE:
                    keep.append(ins)
                    continue
                tn = type(ins).__name__
                if tn == "InstLdweights":
                    ap = ins.ins[0]
                    sig = (getattr(ap, "memref", None),
                           getattr(ap, "offset", None),
                           str(getattr(ap, "ap", None)),
                           str(getattr(ap, "dtype", None)),
                           ins.is_transpose, ins.perf_mode,
                           str(getattr(ins, "tile_position", None)),
                           str(getattr(ins, "tile_size", None)))
                    if sig == last_sig and sig[0] is not None:
                        si = ins.sync_info
                        if si is not None and (list(si.on_wait) or
                                               list(si.on_update)):
                            pending = si
                        removed += 1
                        continue
                    last_sig = sig
                    keep.append(ins)
                elif tn == "InstMatmult":
                    if pending is not None:
                        cur = ins.sync_info
                        cw = list(cur.on_wait) if cur is not None else []
                        cu = list(cur.on_update) if cur is not None else []
                        ins.sync_info = mybir.SyncInfo(
                            on_wait=cw + list(pending.on_wait),
                            on_update=cu + list(pending.on_update))
                        pending = None
                    keep.append(ins)
                else:
                    last_sig = None
                    keep.append(ins)
            assert pending is None
            if len(keep) != len(insts):
                blk.instructions = keep
    return removed


def _build_program():
    nc = bacc.Bacc("TRN2", target_bir_lowering=False, debug=False,
                   num_devices=N_CORES)

    def din(name, shape, dt):
        return nc.dram_tensor(name, list(shape), dt, kind="ExternalInput").ap()

    def dout(name, shape, dt):
        return nc.dram_tensor(name, list(shape), dt, kind="ExternalOutput").ap()

    i_xh = din("xh", [D, T_CORE], BF16)       # bf16 hi of x, transposed
    i_xl = din("xl", [D, T_CORE], BF16)       # bf16 lo of x, transposed
    i_eps = din("eps", [128, N_J, 64], F32)   # eps, partition-major
    i_wh = din("wh", [D, E], BF16)
    i_wl = din("wl", [D, E], BF16)
    i_ws = din("ws", [D, 1], BF16)            # skip weight (bf16 hi)
    i_bias = din("bias", [128, N_BLK, E], F32)  # router|noise bias, replicated
    i_ident = din("ident", [128, 128], F32)
    i_nbsk = din("nbsk", [128, 1], F32)       # -b_skip, replicated

    o_rout = dout("o_rout", [128, N_J, 64], F32)          # [p, J, e]
    o_idx = dout("o_idx", [128, N_J, TOPK], I32)          # [p, J, k]
    o_skip = dout("o_skip", [128, N_GROUPS, N_BLK], F32)  # [p, g, f]

    NSUB = 4
    KSUB = KC // NSUB

    with tile.TileContext(nc) as tc:
        with tc.tile_pool(name="cst", bufs=1) as cst, \
             tc.tile_pool(name="xin", bufs=3) as xin, \
             tc.tile_pool(name="wrk", bufs=2) as wrk, \
             tc.tile_pool(name="drm", bufs=1, space="DRAM") as drm, \
             tc.tile_pool(name="mps", bufs=3, space="PSUM") as mps, \
             tc.tile_pool(name="tps", bufs=2, space="PSUM") as tps, \
             tc.tile_pool(name="sps", bufs=2, space="PSUM") as sps:

            def load_group(g, interleave=None):
                """Queue this group's x hi/lo sub-slabs + eps on the SP
                HWDGE queue (inputs only — outputs go via the ACT queue so
                they never block the x stream)."""
                t0 = g * G_TOK
                xh_t, xl_t = [], []
                for s_ in range(NSUB):
                    k0 = s_ * KSUB
                    xh_s = xin.tile([128, KSUB, G_TOK], BF16,
                                    name=f"xh_s{s_}", tag=f"xh_s{s_}")
                    nc.sync.dma_start(
                        xh_s[:], i_xh[k0 * 128:(k0 + KSUB) * 128,
                                      t0:t0 + G_TOK].rearrange(
                            "(kc p) t -> p kc t", p=128))
                    xl_s = xin.tile([128, KSUB, G_TOK], BF16,
                                    name=f"xl_s{s_}", tag=f"xl_s{s_}")
                    nc.sync.dma_start(
                        xl_s[:], i_xl[k0 * 128:(k0 + KSUB) * 128,
                                      t0:t0 + G_TOK].rearrange(
                            "(kc p) t -> p kc t", p=128))
                    xh_t.append(xh_s)
                    xl_t.append(xl_s)
                    if s_ == 0 and interleave is not None:
                        interleave()
                eps_g = xin.tile([128, N_BLK, 64], F32, name="eps_g",
                                 tag="eps_g")
                nc.sync.dma_start(eps_g[:], i_eps[:, 4 * g:4 * g + N_BLK, :])
                return xh_t, xl_t, eps_g

            # ---- constants; only wh is needed before the first matmul, so
            # group 0's x DMAs are queued right after it ----
            wh_sb = cst.tile([128, KC, E], BF16, name="wh_sb")

            def _load_wh_rest():
                for s_ in range(1, NSUB):
                    k0 = s_ * KSUB
                    nc.sync.dma_start(
                        wh_sb[:, k0:k0 + KSUB, :],
                        i_wh[k0 * 128:(k0 + KSUB) * 128, :].rearrange(
                            "(kc p) e -> p kc e", p=128))

            nc.sync.dma_start(
                wh_sb[:, 0:KSUB, :],
                i_wh[0:KSUB * 128, :].rearrange("(kc p) e -> p kc e", p=128))
            g0_tiles = load_group(0, interleave=_load_wh_rest)
            wl_sb = cst.tile([128, KC, E], BF16, name="wl_sb")
            nc.gpsimd.dma_start(wl_sb[:],
                                i_wl.rearrange("(kc p) e -> p kc e", p=128))
            ws_sb = cst.tile([128, KC, 1], BF16, name="ws_sb")
            nc.gpsimd.dma_start(ws_sb[:],
                                i_ws.rearrange("(kc p) e -> p kc e", p=128))
            bias_sb = cst.tile([128, N_BLK, E], F32, name="bias_sb")
            nc.gpsimd.dma_start(bias_sb[:], i_bias)
            id_sb = cst.tile([128, 128], F32, name="id_sb")
            nc.gpsimd.dma_start(id_sb[:], i_ident)
            nbsk_sb = cst.tile([128, 1], F32, name="nbsk_sb")
            nc.gpsimd.dma_start(nbsk_sb[:], i_nbsk)

            skz_dram = drm.tile([N_GROUPS, G_TOK], F32, name="skz_dram")

            for g in range(N_GROUPS):
                xh_t, xl_t, eps_g = g0_tiles if g == 0 else load_group(g)

                # ---- matmuls: out[e, t] over 3 bf16 terms, kc-major with
                # (wh:xh, wh:xl) adjacent so the duplicate LDWEIGHTS of the
                # second matmul is dropped by _dedup_ldweights ----
                mm_ps = mps.tile([128, G_TOK], F32, name="mm_ps", tag="mm_ps")
                sk_ps = sps.tile([128, G_TOK], F32, name="sk_ps", tag="sk_ps")
                n_mm = 3 * KC
                i_mm = 0
                for kc in range(KC):
                    xh_kc = xh_t[kc // KSUB][:, kc % KSUB, :]
                    xl_kc = xl_t[kc // KSUB][:, kc % KSUB, :]
                    for w_t, x_t in ((wh_sb, xh_kc), (wh_sb, xl_kc)):
                        nc.tensor.matmul(mm_ps[:], lhsT=w_t[:, kc, :], rhs=x_t,
                                         start=(i_mm == 0),
                                         stop=(i_mm == n_mm - 1))
                        i_mm += 1
                # wl pass after all wh pairs: wl_sb is queued behind group
                # 0's x slabs, so the first ~32 matmuls must not need it
                for kc in range(KC):
                    xh_kc = xh_t[kc // KSUB][:, kc % KSUB, :]
                    nc.tensor.matmul(mm_ps[:], lhsT=wl_sb[:, kc, :], rhs=xh_kc,
                                     start=(i_mm == 0),
                                     stop=(i_mm == n_mm - 1))
                    i_mm += 1
                # skip logit (1-term bf16), column-tiled: four concurrent
                # M=1 matmuls in distinct 32-col PE strips, one per
                # 128-token block, so the N=512 stream takes ~N/4 cycles
                for kc in range(KC):
                    xh_kc = xh_t[kc // KSUB][:, kc % KSUB, :]
                    for j in range(N_BLK):
                        nc.tensor.matmul(
                            sk_ps[32 * j:32 * j + 1, j * 128:(j + 1) * 128],
                            lhsT=ws_sb[:, kc, :],
                            rhs=xh_kc[:, j * 128:(j + 1) * 128],
                            start=(kc == 0), stop=(kc == KC - 1),
                            tile_position=(0, 32 * j))

                # ---- PSUM -> SBUF, then PE-transpose back to [t, e] ----
                res_sb = wrk.tile([128, G_TOK], F32, name="res_sb",
                                  tag="res_sb")
                nc.scalar.activation(res_sb[:], mm_ps[:], AF.Copy)
                skg_sb = wrk.tile([1, G_TOK], F32, name="skg_sb", tag="skg_sb")
                for j in range(N_BLK):
                    nc.scalar.activation(
                        skg_sb[:, j * 128:(j + 1) * 128],
                        sk_ps[32 * j:32 * j + 1, j * 128:(j + 1) * 128],
                        AF.Copy)

                # ---- skip prob for this group: bounce [1, 512] through
                # DRAM to [128, 4] lanes, then sigmoid = 1/(1+exp(-z-b)) ----
                nc.gpsimd.dma_start(skz_dram[g:g + 1, :], skg_sb[:])
                sk4_sb = wrk.tile([128, N_BLK], F32, name="sk4_sb",
                                  tag="sk4_sb")
                nc.gpsimd.dma_start(
                    sk4_sb[:],
                    skz_dram[g:g + 1, :].rearrange("o (p f) -> (o p) f",
                                                   p=128))
                ske_sb = wrk.tile([128, N_BLK], F32, name="ske_sb",
                                  tag="ske_sb")
                nc.scalar.activation(ske_sb[:], sk4_sb[:], AF.Exp, scale=-1.0,
                                     bias=nbsk_sb[:])
                nc.vector.tensor_scalar_add(ske_sb[:], ske_sb[:], 1.0)
                skp_sb = wrk.tile([128, N_BLK], F32, name="skp_sb",
                                  tag="skp_sb")
                nc.vector.reciprocal(skp_sb[:], ske_sb[:])
                nc.gpsimd.dma_start(o_skip[:, g, :], skp_sb[:])


                tr_ps = tps.tile([128, N_BLK, 128], F32, name="tr_ps",
                                 tag="tr_ps")
                for j in range(N_BLK):
                    nc.tensor.transpose(tr_ps[:, j, :],
                                        res_sb[:, j * 128:(j + 1) * 128],
                                        id_sb[:])

                # ---- bias add (also moves PSUM -> SBUF) ----
                z_sb = wrk.tile([128, N_BLK, E], F32, name="z_sb", tag="z_sb")
                nc.vector.tensor_add(z_sb[:], tr_ps[:], bias_sb[:])

                # ---- softplus(noise) = Ln(exp(z) + 1) ----
                et_sb = wrk.tile([128, N_BLK, 64], F32, name="et_sb",
                                 tag="et_sb")
                nc.scalar.activation(et_sb[:], z_sb[:, :, 64:128], AF.Exp)
                sp_sb = wrk.tile([128, N_BLK, 64], F32, name="sp_sb",
                                 tag="sp_sb")
                nc.scalar.activation(sp_sb[:], et_sb[:], AF.Ln, bias=1.0)

                # ---- noisy = logits + eps * softplus ----
                ns_sb = wrk.tile([128, N_BLK, 64], F32, name="ns_sb",
                                 tag="ns_sb")
                nc.vector.tensor_mul(ns_sb[:], eps_g[:], sp_sb[:])
                nc.vector.tensor_add(ns_sb[:], ns_sb[:], z_sb[:, :, 0:64])

                # ---- stable top-8 (descending values + indices) ----
                mx_sb = wrk.tile([128, N_BLK, TOPK], F32, name="mx_sb",
                                 tag="mx_sb")
                ix_sb = wrk.tile([128, N_BLK, TOPK], I32, name="ix_sb",
                                 tag="ix_sb")
                for j in range(N_BLK):
                    nc.vector.max(mx_sb[:, j, :], ns_sb[:, j, :])
                    nc.vector.max_index(ix_sb[:, j, :].bitcast(U32),
                                        mx_sb[:, j, :], ns_sb[:, j, :])
                nc.gpsimd.dma_start(o_idx[:, 4 * g:4 * g + N_BLK, :], ix_sb[:])

                # ---- gates: exp(v - m0) masked by v >= t8, normalized ----
                nm0_sb = wrk.tile([128, N_BLK], F32, name="nm0_sb",
                                  tag="nm0_sb")
                nc.vector.tensor_scalar_mul(nm0_sb[:], mx_sb[:, :, 0], -1.0)
                ex_sb = wrk.tile([128, N_BLK, 64], F32, name="ex_sb",
                                 tag="ex_sb")
                for j in range(N_BLK):
                    nc.scalar.activation(ex_sb[:, j, :], ns_sb[:, j, :],
                                         AF.Exp, bias=nm0_sb[:, j:j + 1])
                rs_sb = wrk.tile([128, N_BLK], F32, name="rs_sb", tag="rs_sb")
                r_sb = wrk.tile([128, N_BLK, 64], F32, name="r_sb", tag="r_sb")
                for j in range(N_BLK):
                    nc.vector.scalar_tensor_tensor(
                        r_sb[:, j, :], ns_sb[:, j, :], mx_sb[:, j, 7:8],
                        ex_sb[:, j, :], op0=OP.is_ge, op1=OP.mult,
                        accum_out=rs_sb[:, j:j + 1])
                ri_sb = wrk.tile([128, N_BLK], F32, name="ri_sb", tag="ri_sb")
                nc.vector.reciprocal(ri_sb[:], rs_sb[:])
                rg_sb = wrk.tile([128, N_BLK, 64], F32, name="rg_sb",
                                 tag="rg_sb")
                for j in range(N_BLK):
                    nc.vector.tensor_scalar_mul(
                        rg_sb[:, j, :], r_sb[:, j, :], ri_sb[:, j:j + 1])
                nc.gpsimd.dma_start(o_rout[:, 4 * g:4 * g + N_BLK, :],
                                    rg_sb[:])
    # Pin every activation to act table 6 (natural_log_exp_and_others: has
    # Copy+Exp+Ln) so a single ACT_TABLE_LOAD serves the whole kernel. The
    # default chooser pairs Exp with table 0 and Ln with table 6 and
    # reloads ~1.3us on every switch. Indices must stay aligned with
    # act_info.json, so earlier entries are emptied rather than removed.
    import concourse.bacc as _bacc_mod
    from concourse.hw_specs import get_activation_tables as _gat
    real = list(_gat("gen3").items())
    pinned = {}
    for i, (name, funcs) in enumerate(real):
        pinned[name] = funcs if name == "natural_log_exp_and_others" else set()
    orig_gat = _bacc_mod.get_activation_tables
    _bacc_mod.get_activation_tables = lambda arch: pinned
    try:
        nc.compile()
    finally:
        _bacc_mod.get_activation_tables = orig_gat
    n_removed = _dedup_ldweights(nc)
    assert n_removed >= N_GROUPS * KC * 3 // 4, n_removed
    return nc


def _get_program():
    global _PROGRAM
    if _PROGRAM is None:
        _PROGRAM = _build_program()
    return _PROGRAM


def kernel(x, eps, w_router, b_router, w_noise, b_noise, w_skip, b_skip):
    nc = _get_program()

    x = np.ascontiguousarray(np.asarray(x, dtype=np.float32)).reshape(-1, D)
    eps = np.ascontiguousarray(np.asarray(eps, dtype=np.float32)).reshape(-1, 64)
    n_tok = x.shape[0]
    assert n_tok == N_CORES * T_CORE

    W = np.concatenate([np.asarray(w_router, np.float32),
                        np.asarray(w_noise, np.float32)], axis=1)
    wh = W.astype(BF)
    wl = (W - wh.astype(np.float32)).astype(BF)
    ws = np.asarray(w_skip, np.float32).astype(BF)
    bias_cat = np.concatenate([np.asarray(b_router, np.float32),
                               np.asarray(b_noise, np.float32)])
    bias_rep = np.broadcast_to(bias_cat, (128, N_BLK, E)).copy()
    ident = np.eye(128, dtype=np.float32)
    nbsk = np.full((128, 1), -float(np.asarray(b_skip).reshape(())), np.float32)

    xh_full = x.astype(BF)
    xl_full = (x - xh_full.astype(np.float32)).astype(BF)

    in_maps = []
    for c in range(N_CORES):
        s = slice(c * T_CORE, (c + 1) * T_CORE)
        in_maps.append({
            "xh": np.ascontiguousarray(xh_full[s].T),
            "xl": np.ascontiguousarray(xl_full[s].T),
            # eps token t = J*128 + p  ->  [p, J, e]
            "eps": np.ascontiguousarray(
                eps[s].reshape(N_J, 128, 64).transpose(1, 0, 2)),
            "wh": wh, "wl": wl, "ws": ws,
            "bias": bias_rep, "ident": ident, "nbsk": nbsk,
        })

    res = bass_utils.run_bass_kernel_spmd(nc, in_maps,
                                          core_ids=list(range(N_CORES)))
    global LAST_RESULTS
    LAST_RESULTS = res

    router = np.empty((n_tok, 64), np.float32)
    indices = np.empty((n_tok, TOPK), np.int32)
    skip = np.empty((n_tok,), np.float32)
    for c in range(N_CORES):
        out = res.results[c]
        s = slice(c * T_CORE, (c + 1) * T_CORE)
        # [p, J, ...] -> token t = J*128 + p
        router[s] = out["o_rout"].transpose(1, 0, 2).reshape(T_CORE, 64)
        indices[s] = out["o_idx"].transpose(1, 0, 2).reshape(T_CORE, TOPK)
        # skip output is [p, g, f] with token = g*512 + p*4 + f
        skip[s] = out["o_skip"].transpose(1, 0, 2).reshape(T_CORE)

    B, S = 4, 8192
    return (router.reshape(B, S, 64), indices.reshape(B, S, TOPK),
            skip.reshape(B, S, 1).astype(np.float32))
